# revision 2
# baseline (speedup 1.0000x reference)
"""CommNet message-passing kernel for Trainium2 (8 NeuronCores) — v6.

v3 made every DMA fully contiguous per partition (host lays per-core
arrays out in consumption order). v4 additionally ships rnn/obs to the
device as bfloat16 and stores out as bfloat16 (host up-converts), halving
HBM traffic: 48MB/core instead of 96MB. The 2e-2 rel-err budget dwarfs
bf16 round-off (~5e-3 end to end). The whole compute pipe runs bf16
(DVE/Act at 2x 16-bit rate, PE transposes+GEMM at 1 cycle/col), PSUM
accumulation stays f32.

Layouts (token s = p*nj + q; p = partition, q = tile column):
  - rnn  dram bf16 [b, p, q, a, h]: one 16KB run/partition per b-slab
  - obs  dram bf16 [b, p, a, q, h]: one 16KB run/partition per b-slab
  - out  dram bf16 [b, p, a, q, h]: one 16KB run/partition per b-store
~3k descriptors/core total vs 196k in the 512B f32 baseline.
"""

import os
import sys

import numpy as np

for _p in ("/opt/trn_rl_repo", "/root/.axon_site/_ro/trn_rl_repo"):
    if os.path.isdir(_p) and _p not in sys.path:
        sys.path.append(_p)

A, B, S, H = 8, 64, 1024, 128
NCORES = 8
BLOC = B // NCORES  # 8 env batches per core


def _build_program(s_len=S, transpose_dt="bfloat16", reps=1):
    """Build the per-core Bass program (identical on all cores).

    reps>1 repeats the whole main loop (same I/O) — timing only."""
    import concourse.bass as bass  # noqa: F401
    import concourse.bacc as bacc
    import concourse.tile as tile
    from concourse import mybir

    f32 = mybir.dt.float32
    bf16 = mybir.dt.bfloat16
    i32 = mybir.dt.int32

    assert s_len % 512 == 0
    nj = s_len // 128       # tokens per partition; tile-column count
    ngroups = s_len // 512  # 512-token groups per (a,b)

    nc = bacc.Bacc("TRN2", target_bir_lowering=False, debug=False,
                   num_devices=NCORES)

    rnn = nc.dram_tensor("rnn", [BLOC, 128, nj, A, H], bf16,
                         kind="ExternalInput").ap()
    obs = nc.dram_tensor("obs", [BLOC, 128, A, nj, H], bf16,
                         kind="ExternalInput").ap()
    alive = nc.dram_tensor("alive", [A, BLOC, s_len], i32,
                           kind="ExternalInput").ap()
    wt = nc.dram_tensor("wt", [H, H], f32, kind="ExternalInput").ap()
    bias = nc.dram_tensor("bias", [H, 1], f32, kind="ExternalInput").ap()
    ident = nc.dram_tensor("ident", [128, 128], f32, kind="ExternalInput").ap()
    sel = nc.dram_tensor("sel", [64, 8], f32, kind="ExternalInput").ap()
    sel2 = nc.dram_tensor("sel2", [8, 64], f32, kind="ExternalInput").ap()
    out = nc.dram_tensor("out", [BLOC, 128, A, nj, H], bf16,
                         kind="ExternalOutput").ap()

    alive_r = alive.rearrange("a b s -> (a b) s")

    tdt = bf16
    mm_dt = bf16

    with tile.TileContext(nc) as tc:
        with tc.tile_pool(name="consts", bufs=1) as consts, \
             tc.tile_pool(name="pre", bufs=1) as pre, \
             tc.tile_pool(name="prepsum", bufs=1, space="PSUM") as prepsum, \
             tc.tile_pool(name="rnnp", bufs=3) as rnn_pool, \
             tc.tile_pool(name="obsp", bufs=3) as obs_pool, \
             tc.tile_pool(name="outp", bufs=3) as out_pool, \
             tc.tile_pool(name="scaledp", bufs=4) as scaled_pool, \
             tc.tile_pool(name="mtp", bufs=4) as mt_pool, \
             tc.tile_pool(name="obp", bufs=4) as ob_pool, \
             tc.tile_pool(name="pap", bufs=2, space="PSUM") as pa_pool, \
             tc.tile_pool(name="pbp", bufs=2, space="PSUM") as pb_pool, \
             tc.tile_pool(name="pcp", bufs=3, space="PSUM") as pc_pool:

            # ---- constants ----
            wt_sb = consts.tile([128, 128], f32, tag="wt")
            nc.sync.dma_start(out=wt_sb, in_=wt)
            wt_r = consts.tile([128, 128], mm_dt, tag="wtr")
            nc.vector.tensor_copy(out=wt_r, in_=wt_sb)
            id_sb = consts.tile([128, 128], f32, tag="id")
            nc.sync.dma_start(out=id_sb, in_=ident)
            b_sb = consts.tile([128, 1], f32, tag="b")
            nc.sync.dma_start(out=b_sb, in_=bias)
            sel_sb = consts.tile([64, 8], f32, tag="sel")
            nc.sync.dma_start(out=sel_sb, in_=sel)
            sel2_sb = consts.tile([8, 64], f32, tag="sel2")
            nc.sync.dma_start(out=sel2_sb, in_=sel2)
            id_t = consts.tile([128, 128], tdt, tag="idt")
            nc.vector.tensor_copy(out=id_t, in_=id_sb)

            # ---- scale = alive / max(sum_a alive, 1) ----
            alive_i = pre.tile([64, s_len], i32, tag="alive_i")
            nc.sync.dma_start(out=alive_i, in_=alive_r)
            alive_f = pre.tile([64, s_len], f32, tag="alive_f")
            nc.vector.tensor_copy(out=alive_f, in_=alive_i)

            denom = pre.tile([8, s_len], f32, tag="denom")
            for hh in range(s_len // 512):
                dps = prepsum.tile([8, 512], f32, tag="pp")
                nc.tensor.matmul(out=dps, lhsT=sel_sb,
                                 rhs=alive_f[:, 512 * hh:512 * (hh + 1)],
                                 start=True, stop=True)
                nc.vector.tensor_scalar_max(
                    out=denom[:, 512 * hh:512 * (hh + 1)], in0=dps,
                    scalar1=1.0)
            inv = pre.tile([8, s_len], f32, tag="inv")
            nc.vector.reciprocal(out=inv, in_=denom)

            scale_nat = pre.tile([64, 128, nj], f32, tag="scale_nat")
            scale_fl = scale_nat.rearrange("ab p q -> ab (p q)")
            for hh in range(s_len // 512):
                ips = prepsum.tile([64, 512], f32, tag="pp")
                nc.tensor.matmul(out=ips, lhsT=sel2_sb,
                                 rhs=inv[:, 512 * hh:512 * (hh + 1)],
                                 start=True, stop=True)
                nc.vector.tensor_mul(
                    out=scale_fl[:, 512 * hh:512 * (hh + 1)],
                    in0=alive_f[:, 512 * hh:512 * (hh + 1)], in1=ips)

            # scale_sb[p, 64*q + ab] = scale for token (a, b, p*nj+q)
            scps = prepsum.tile([128, 64 * nj], f32, tag="pp")
            for q in range(nj):
                nc.tensor.matmul(out=scps[:, 64 * q:64 * (q + 1)],
                                 lhsT=scale_nat[:, :, q],
                                 rhs=id_sb[:64, :64], is_transpose=True,
                                 start=(q == 0), stop=(q == nj - 1))
            scale_sb = pre.tile([128, 64 * nj], f32, tag="scale_sb")
            nc.vector.tensor_copy(out=scale_sb, in_=scps)

            # ---- main loop: per local batch b, rnn/obs slabs; per a, GEMM ----
            ident_f = mybir.ActivationFunctionType.Identity
            for _rep in range(reps):
              for b in range(BLOC):
                rnn_t = rnn_pool.tile([128, nj, A, H], bf16, tag="rnn_t")
                nc.sync.dma_start(out=rnn_t, in_=rnn[b])
                obs_t = obs_pool.tile([128, A, nj, H], bf16, tag="obs_t")
                nc.sync.dma_start(out=obs_t, in_=obs[b])
                out_t = out_pool.tile([128, A, nj, H], bf16, tag="out_t")
                obs_fl = obs_t.rearrange("p a q h -> p a (q h)")
                out_fl = out_t.rearrange("p a q h -> p a (q h)")
                for a in range(A):
                    ab = a * 8 + b
                    for g in range(ngroups):
                        scaled = scaled_pool.tile([128, 4, 128], tdt,
                                                  tag="scaled")
                        for jj in range(4):
                            q = 4 * g + jj
                            col = 64 * q + ab
                            nc.vector.tensor_scalar_mul(
                                out=scaled[:, jj, :],
                                in0=rnn_t[:, q, a, :],
                                scalar1=scale_sb[:, col:col + 1])
                        pa = pa_pool.tile([128, 512], tdt, tag="pa")
                        for jj in range(4):
                            nc.tensor.matmul(
                                out=pa[:, 128 * jj:128 * (jj + 1)],
                                lhsT=scaled[:, jj, :],
                                rhs=id_t,
                                is_transpose=True,
                                start=(jj == 0), stop=(jj == 3))
                        mt = mt_pool.tile([128, 512], mm_dt, tag="mt")
                        nc.scalar.copy(out=mt, in_=pa)
                        pb = pb_pool.tile([128, 512], f32, tag="pb")
                        nc.tensor.matmul(out=pb, lhsT=wt_r, rhs=mt,
                                         start=True, stop=True)
                        ob = ob_pool.tile([128, 512], tdt, tag="ob")
                        nc.scalar.activation(out=ob, in_=pb, func=ident_f,
                                             bias=b_sb, scale=1.0)
                        pc = pc_pool.tile([128, 512], tdt, tag="pc")
                        for jj in range(4):
                            nc.tensor.matmul(
                                out=pc[:, 128 * jj:128 * (jj + 1)],
                                lhsT=ob[:, 128 * jj:128 * (jj + 1)],
                                rhs=id_t,
                                is_transpose=True,
                                start=(jj == 0), stop=(jj == 3))
                        nc.vector.tensor_add(
                            out=out_fl[:, a, 512 * g:512 * (g + 1)],
                            in0=pc,
                            in1=obs_fl[:, a, 512 * g:512 * (g + 1)])
                nc.scalar.dma_start(out=out[b], in_=out_t)
    nc.compile()
    return nc


def make_in_maps(obs, rnn_h, alive, W, b, s_len=S):
    """Shard + permute + bf16-convert inputs into per-core arrays."""
    import ml_dtypes
    bf16 = ml_dtypes.bfloat16
    nj = s_len // 128
    obs4 = obs.reshape(A, B, S, H)
    wt = np.ascontiguousarray(W.T.astype(np.float32))
    b2 = np.ascontiguousarray(b.astype(np.float32).reshape(H, 1))
    ident = np.eye(128, dtype=np.float32)
    sel = np.zeros((64, 8), np.float32)
    sel[np.arange(64), np.arange(64) % 8] = 1.0
    sel2 = np.ascontiguousarray(sel.T)
    in_maps = []
    for c in range(NCORES):
        bs = slice(BLOC * c, BLOC * (c + 1))
        # rnn: [s, b, a, h] -> [b, p, q, a, h], s = p*nj + q
        rc = rnn_h[:s_len, bs].reshape(128, nj, BLOC, A, H)
        rc = np.ascontiguousarray(rc.transpose(2, 0, 1, 3, 4).astype(bf16))
        # obs: [a, b, s, h] -> [b, p, a, q, h]
        oc = obs4[:, bs, :s_len].reshape(A, BLOC, 128, nj, H)
        oc = np.ascontiguousarray(oc.transpose(1, 2, 0, 3, 4).astype(bf16))
        in_maps.append({
            "rnn": rc,
            "obs": oc,
            "alive": np.ascontiguousarray(alive[:, bs, :s_len, 0]),
            "wt": wt, "bias": b2, "ident": ident, "sel": sel, "sel2": sel2,
        })
    return in_maps


def unshard_out(res, s_len=S):
    """Invert the out permute: per-core [b, p, a, q, h] -> (A*B, S, H) f32."""
    full = np.empty((A, B, s_len, H), np.float32)
    for c in range(NCORES):
        oc = np.asarray(res[c]["out"]).astype(np.float32)
        oc = oc.transpose(2, 0, 1, 3, 4).reshape(A, BLOC, s_len, H)
        full[:, BLOC * c:BLOC * (c + 1)] = oc
    return full.reshape(A * B, s_len, H)


_NC_CACHE = {}


def get_nc(s_len=S, transpose_dt="bfloat16", reps=1):
    key = (s_len, transpose_dt, reps)
    if key not in _NC_CACHE:
        _NC_CACHE[key] = _build_program(s_len, transpose_dt, reps)
    return _NC_CACHE[key]


DEFAULT_TRANSPOSE_DT = "bfloat16"


def kernel(obs, rnn_h, alive, W, b):
    from concourse.bass_utils import run_bass_kernel_spmd

    nc = get_nc(S, DEFAULT_TRANSPOSE_DT)
    in_maps = make_in_maps(obs, rnn_h, alive, W, b)
    res = run_bass_kernel_spmd(nc, in_maps, list(range(NCORES))).results
    return unshard_out(res)
